# revision 2
# baseline (speedup 1.0000x reference)
"""Trainium2 kernel for nn_ConservativePotential (GNN message passing).

Strategy: target-node sharding across 8 NeuronCores. Host rebalances each
core's 2560-node slice into 128-node blocks (permutation baked into shipped
indices). Device per layer: dma_gather of node rows [s|v] from an HBM table,
gate matmuls (feat^T stationary, folded bf16 weights), DVE message formation,
one-hot scatter matmuls accumulating per (block,set) in PSUM with segment-mean
scaling, node MLP + LayerNorm via PE transposes, contiguous write-back, and a
cross-core AllGather feeding the next layer's gathers. Readout (gated equiv
block + per-node energy) on device; host does the final per-graph sum.

Everything below is self-contained (no sibling imports).
"""
import numpy as np
import ml_dtypes
from dataclasses import dataclass

SD, VD = 64, 16
RBF_DIM, ED, TD = 16, 16, 64
CUTOFF = 10.0
L = 5
N_CORES = 8
ROW = 128
MSG = 112
BF16 = ml_dtypes.bfloat16
FP8 = ml_dtypes.float8_e4m3

LAST_HW_EXEC_NS = None


@dataclass
class Cfg:
    NB: int
    TL: int
    TG: int
    CH: int = 16
    L: int = L

    @property
    def npc(self):
        return self.NB * 128

    @property
    def npad(self):
        return self.npc * N_CORES

    @property
    def tpb(self):
        return self.TL + self.TG

    @property
    def n_tiles(self):
        return self.NB * self.tpb

    @property
    def epc(self):
        return self.n_tiles * 128

    @property
    def n_chunks(self):
        assert self.n_tiles % self.CH == 0
        return self.n_tiles // self.CH

    def set_of_tile(self, t):
        return 0 if (t % self.tpb) < self.TL else 1

    def block_of_tile(self, t):
        return t // self.tpb


def _embed_maxnorm(table, idx):
    n = np.linalg.norm(table, axis=-1, keepdims=True)
    t = table * np.minimum(1.0, 1.0 / np.maximum(n, 1e-12))
    return t[idx]


def _silu(x):
    return x / (1.0 + np.exp(-x))


def _rbf(d):
    centers = np.linspace(0.0, CUTOFF, RBF_DIM, dtype=np.float32)
    gamma = np.float32((RBF_DIM / CUTOFF) ** 2)
    env = 0.5 * (np.cos(np.pi * np.clip(d / CUTOFF, 0.0, 1.0)) + 1.0)
    return (np.exp(-gamma * (d[:, None] - centers) ** 2) * env[:, None]).astype(np.float32)


def _wrap16(idx):
    n = len(idx)
    w = np.asarray(idx, np.int16).reshape(n // 16, 16).T
    return np.ascontiguousarray(np.tile(w, (8, 1)))


def pick_cfg(inputs):
    NB = 20
    npc = NB * 128
    N = np.asarray(inputs["x"]).shape[0]
    deg = [np.zeros(N, np.int64), np.zeros(N, np.int64)]
    for si, key in enumerate(["edge_index_local", "edge_index_global"]):
        tgt = np.asarray(inputs[key])[1]
        np.add.at(deg[si], tgt, 1)
    caps = []
    for si in range(2):
        worst = 0
        for c in range(N_CORES):
            lo, hi = c * npc, min((c + 1) * npc, N)
            tot = int(deg[si][lo:hi].sum())
            mx = int(deg[si][lo:hi].max()) if hi > lo else 0
            worst = max(worst, (tot + NB - 1) // NB + mx)
        caps.append((worst + 127) // 128)
    TL, TG = caps
    CH = 16
    while (NB * (TL + TG)) % CH != 0:
        TG += 1
    return Cfg(NB=NB, TL=TL, TG=TG, CH=CH)


def prep(inputs, cfg):
    x = np.asarray(inputs["x"]).astype(np.int64)
    t = np.asarray(inputs["t"]).astype(np.float32)
    pos = np.asarray(inputs["pos"]).astype(np.float32)
    batch = np.asarray(inputs["batch"]).astype(np.int64)
    p = {k: np.asarray(inputs[k]).astype(np.float32)
         for k in ["atom_table", "bond_table", "tW1", "tb1", "tW2", "tb2",
                   "We_loc", "be_loc", "We_glob", "be_glob", "Wu1", "bu1",
                   "Wu2", "bu2", "ln_g", "ln_b", "Wv_down", "Wd1", "bd1",
                   "Wd2", "bd2"]}
    N = x.shape[0]
    NPC = cfg.npc

    sets = [
        ("loc", np.asarray(inputs["edge_index_local"]).astype(np.int64),
         np.asarray(inputs["edge_attr_local"]).astype(np.int64)),
        ("glob", np.asarray(inputs["edge_index_global"]).astype(np.int64),
         np.asarray(inputs["edge_attr_global"]).astype(np.int64)),
    ]

    s0 = _embed_maxnorm(p["atom_table"], x)
    temb = _silu(_silu(t @ p["tW1"] + p["tb1"]) @ p["tW2"] + p["tb2"])
    s0 = (s0 + temb[batch]).astype(np.float32)

    deg = np.zeros(N, np.int64)
    for _, ei, _ in sets:
        np.add.at(deg, ei[1], 1)
    permpos = np.zeros(N, np.int64)
    for c in range(N_CORES):
        lo, hi = c * NPC, min((c + 1) * NPC, N)
        if hi <= lo:
            continue
        nodes = np.arange(lo, hi)
        order = nodes[np.argsort(-deg[nodes], kind="stable")]
        binw = np.zeros(cfg.NB, np.int64)
        bincnt = np.zeros(cfg.NB, np.int64)
        for n in order:
            ok = np.where(bincnt < 128)[0]
            b = ok[np.argmin(binw[ok])]
            permpos[n] = c * NPC + b * 128 + bincnt[b]
            binw[b] += deg[n]
            bincnt[b] += 1
    origin = np.full(cfg.npad, -1, np.int64)
    origin[permpos] = np.arange(N)

    set_data = []
    for name, ei, ea in sets:
        src, tgt = ei[0], ei[1]
        r = pos[tgt] - pos[src]
        d = np.sqrt(np.clip(np.sum(r * r, -1), 1e-6, None)).astype(np.float32)
        rn = (r / d[:, None]).astype(np.float32)
        feat = np.zeros((len(src), 24), np.float32)
        feat[:, 0:16] = _rbf(d)
        feat[np.arange(len(src)), 16 + ea] = 1.0
        feat[:, 21] = 1.0
        set_data.append((name, src, tgt, feat, rn))

    bond_n = _embed_maxnorm(p["bond_table"], np.arange(p["bond_table"].shape[0]))
    wfold = np.zeros((2, cfg.L, 24, 160), np.float32)
    for si, (Wel, bel) in enumerate([(p["We_loc"], p["be_loc"]),
                                     (p["We_glob"], p["be_glob"])]):
        for l in range(cfg.L):
            Wf = np.zeros((24, SD + 2 * VD), np.float32)
            Wf[0:16] = Wel[l][:RBF_DIM]
            Wf[16:16 + bond_n.shape[0]] = bond_n @ Wel[l][RBF_DIM:]
            Wf[21] = bel[l]
            wfold[si, l, :, 0:64] = Wf[:, 0:SD]
            for i in range(3):
                wfold[si, l, :, 64 + i * 16:80 + i * 16] = Wf[:, SD:SD + VD]
                wfold[si, l, :, 112 + i * 16:128 + i * 16] = Wf[:, SD + VD:]

    core_inputs = []
    for c in range(N_CORES):
        src_stream = np.zeros(cfg.epc, np.int64)
        featT = np.zeros((24, cfg.epc), np.float32)
        rn_stream = np.zeros((cfg.epc, 3), np.float32)
        T_onehot = np.zeros((128, cfg.n_tiles, 128), np.float32)
        invcnt = np.ones((128, cfg.NB, 2), np.float32)

        for si, (name, src, tgt, feat, rn) in enumerate(set_data):
            gpos_t = permpos[tgt]
            es = np.where((gpos_t // NPC) == c)[0]
            lpos = gpos_t[es] - c * NPC
            blk, slot = lpos // 128, lpos % 128
            TS = cfg.TL if si == 0 else cfg.TG
            base_t = 0 if si == 0 else cfg.TL
            cnt = np.zeros((cfg.NB, 128), np.int64)
            np.add.at(cnt, (blk, slot), 1)
            invcnt[:, :, si] = (1.0 / np.clip(cnt, 1, None)).T
            for b in range(cfg.NB):
                sel = blk == b
                eb, sb = es[sel], slot[sel]
                if len(eb) > TS * 128:
                    raise RuntimeError("block overflow")
                o = np.argsort(permpos[src[eb]], kind="stable")
                eb, sb = eb[o], sb[o]
                t0 = (b * cfg.tpb + base_t) * 128
                sl = slice(t0, t0 + len(eb))
                src_stream[sl] = permpos[src[eb]]
                featT[:, sl] = feat[eb].T
                rn_stream[sl] = rn[eb]
                el = np.arange(len(eb))
                T_onehot[el % 128, el // 128 + b * cfg.tpb + base_t, sb] = 1.0

        tab0 = np.zeros((cfg.npad, ROW), np.float32)
        tab0[permpos, 0:64] = s0
        own0 = tab0[c * NPC:(c + 1) * NPC]

        ci = {
            "own0": np.ascontiguousarray(own0),
            "srcidx": _wrap16(src_stream),
            "featT": featT.astype(BF16),
            "rn": np.ascontiguousarray(
                rn_stream.reshape(cfg.n_tiles, 128, 3).transpose(1, 0, 2)).astype(BF16),
            "T": np.ascontiguousarray(T_onehot).astype(FP8),
            "invcnt": invcnt,
            "wfold": np.ascontiguousarray(
                wfold.reshape(2 * cfg.L, 24, 160).transpose(1, 0, 2)).astype(BF16),
            "wu1a": np.ascontiguousarray(p["Wu1"][:, 0:64].transpose(1, 0, 2)).astype(BF16),
            "wu1b": np.ascontiguousarray(p["Wu1"][:, 64:128].transpose(1, 0, 2)).astype(BF16),
            "wu2": np.ascontiguousarray(p["Wu2"].transpose(1, 0, 2)).astype(BF16),
            "bu1": np.ascontiguousarray(p["bu1"].T)[:, :, None].astype(np.float32),
            "bu2": np.ascontiguousarray(p["bu2"].T)[:, :, None].astype(np.float32),
            "lng": np.tile(p["ln_g"].T[None], (128, 1, 1)).transpose(0, 2, 1).copy(),
            "lnb": np.tile(p["ln_b"].T[None], (128, 1, 1)).transpose(0, 2, 1).copy(),
            "wv": p["Wv_down"].astype(BF16),
            "wd1a": np.ascontiguousarray(p["Wd1"][0:64]).astype(BF16),
            "wd1b": np.ascontiguousarray(p["Wd1"][64:80]).astype(BF16),
            "bd1": p["bd1"][:, None].astype(np.float32),
            "wd2": p["Wd2"].astype(BF16),
            "bd2": np.float32(p["bd2"]).reshape(1, 1),
            "ident": np.eye(128, dtype=np.float32),
        }
        core_inputs.append(ci)

    host_meta = {"origin": origin, "permpos": permpos, "batch": batch, "N": N}
    return core_inputs, host_meta


def epilogue(e_cores, host_meta, G):
    origin = host_meta["origin"]
    batch = host_meta["batch"]
    e_flat = np.concatenate([np.asarray(e, np.float64).reshape(-1) for e in e_cores])
    out = np.zeros((G, 1), np.float64)
    valid = origin >= 0
    np.add.at(out[:, 0], batch[origin[valid]], e_flat[valid])
    return out.astype(np.float32)


INPUT_KEYS = ["own0", "srcidx", "featT", "rn", "T", "invcnt", "wfold",
              "wu1a", "wu1b", "wu2", "bu1", "bu2", "lng", "lnb", "wv",
              "wd1a", "wd1b", "bd1", "wd2", "bd2", "ident"]


def build(cfg, repeat=1):
    import concourse.bass as bass
    import concourse.tile as tile
    from concourse import bacc, mybir
    from concourse.library_config import mlp
    from concourse.tile import add_dep_helper
    from contextlib import ExitStack

    dt = mybir.dt
    AT = mybir.ActivationFunctionType
    OP = mybir.AluOpType
    NB, TL, TG, CH = cfg.NB, cfg.TL, cfg.TG, cfg.CH
    NPC, NPAD, NT, EPC = cfg.npc, cfg.npad, cfg.n_tiles, cfg.epc
    SC = 8 if CH % 8 == 0 else CH
    n_sub = CH // SC
    LYR = cfg.L
    NCH = 512 if (NB * 128) % 512 == 0 else 128

    nc = bacc.Bacc("TRN2", target_bir_lowering=False)

    def din(name, shape, dtype):
        return nc.dram_tensor(name, shape, dtype, kind="ExternalInput")

    own0 = din("own0", [NPC, ROW], dt.float32)
    srcidx_d = din("srcidx", [128, EPC // 16], dt.int16)
    featT_d = din("featT", [24, EPC], dt.bfloat16)
    rn_d = din("rn", [128, NT, 3], dt.bfloat16)
    T_d = din("T", [128, NT, 128], dt.float8e4)
    invcnt_d = din("invcnt", [128, NB, 2], dt.float32)
    wfold_d = din("wfold", [24, 2 * LYR, 160], dt.bfloat16)
    wu1a_d = din("wu1a", [64, LYR, 64], dt.bfloat16)
    wu1b_d = din("wu1b", [64, LYR, 64], dt.bfloat16)
    wu2_d = din("wu2", [64, LYR, 64], dt.bfloat16)
    bu1_d = din("bu1", [64, LYR, 1], dt.float32)
    bu2_d = din("bu2", [64, LYR, 1], dt.float32)
    lng_d = din("lng", [128, LYR, 64], dt.float32)
    lnb_d = din("lnb", [128, LYR, 64], dt.float32)
    wv_d = din("wv", [16, 16], dt.bfloat16)
    wd1a_d = din("wd1a", [64, 64], dt.bfloat16)
    wd1b_d = din("wd1b", [16, 64], dt.bfloat16)
    bd1_d = din("bd1", [64, 1], dt.float32)
    wd2_d = din("wd2", [64, 1], dt.bfloat16)
    bd2_d = din("bd2", [1, 1], dt.float32)
    ident_d = din("ident", [128, 128], dt.float32)

    e_out = nc.dram_tensor("e", [1, NPC], dt.float32, kind="ExternalOutput")

    cc_in = nc.dram_tensor("cc_in", [NPC, ROW], dt.float32)
    tabs = [nc.dram_tensor(f"tab{l}", [NPAD, ROW], dt.float32, addr_space="Shared")
            for l in range(LYR)]

    with tile.TileContext(nc) as tc, ExitStack() as ctx:
        cpool = ctx.enter_context(tc.tile_pool(name="const", bufs=1))
        ppool = ctx.enter_context(tc.tile_pool(name="persist", bufs=1))
        gpool = ctx.enter_context(tc.tile_pool(name="gath", bufs=3))
        tpool = ctx.enter_context(tc.tile_pool(name="tmat", bufs=3))
        fpool = ctx.enter_context(tc.tile_pool(name="feat", bufs=3))
        mpool = ctx.enter_context(tc.tile_pool(name="msg", bufs=2))
        rpool = ctx.enter_context(tc.tile_pool(name="rnb", bufs=3))
        npool = ctx.enter_context(tc.tile_pool(name="node", bufs=1))
        pgA = ctx.enter_context(tc.tile_pool(name="pgA", bufs=2, space="PSUM"))
        pgB = ctx.enter_context(tc.tile_pool(name="pgB", bufs=2, space="PSUM"))
        pagg = ctx.enter_context(tc.tile_pool(name="pagg", bufs=1, space="PSUM"))
        pnode = ctx.enter_context(tc.tile_pool(name="pnode", bufs=1, space="PSUM"))

        nc.gpsimd.load_library(mlp)

        def cload(dram, shape, dtype):
            t = cpool.tile(shape, dtype, name=f"c_{dram.name}", tag=f"c_{dram.name}")
            nc.sync.dma_start(t[:], dram[:])
            return t

        srcidx = cload(srcidx_d, [128, EPC // 16], dt.int16)
        rn_sb = cload(rn_d, [128, NT, 3], dt.bfloat16)
        invcnt = cload(invcnt_d, [128, NB, 2], dt.float32)
        wfold = cload(wfold_d, [24, 2 * LYR, 160], dt.bfloat16)
        wu1a = cload(wu1a_d, [64, LYR, 64], dt.bfloat16)
        wu1b = cload(wu1b_d, [64, LYR, 64], dt.bfloat16)
        wu2 = cload(wu2_d, [64, LYR, 64], dt.bfloat16)
        bu1 = cload(bu1_d, [64, LYR, 1], dt.float32)
        bu2 = cload(bu2_d, [64, LYR, 1], dt.float32)
        lng = cload(lng_d, [128, LYR, 64], dt.float32)
        lnb = cload(lnb_d, [128, LYR, 64], dt.float32)
        wv = cload(wv_d, [16, 16], dt.bfloat16)
        wd1a = cload(wd1a_d, [64, 64], dt.bfloat16)
        wd1b = cload(wd1b_d, [16, 64], dt.bfloat16)
        bd1 = cload(bd1_d, [64, 1], dt.float32)
        wd2 = cload(wd2_d, [64, 1], dt.bfloat16)
        bd2 = cload(bd2_d, [1, 1], dt.float32)
        idf = cload(ident_d, [128, 128], dt.float32)
        idb = cpool.tile([128, 128], dt.bfloat16, name="idb", tag="idb")
        nc.vector.tensor_copy(idb[:], idf[:])
        c_inv64 = cpool.tile([128, 1], dt.float32, name="c_inv64", tag="c_inv64")
        nc.vector.memset(c_inv64[:], 1.0 / 64.0)
        c_eps5 = cpool.tile([128, 1], dt.float32, name="c_eps5", tag="c_eps5")
        nc.vector.memset(c_eps5[:], 1e-5)
        c_eps8 = cpool.tile([128, 1], dt.float32, name="c_eps8", tag="c_eps8")
        nc.vector.memset(c_eps8[:], 1e-8)

        s_nm = ppool.tile([128, NB, 64], dt.float32)
        v_nm = ppool.tile([128, NB, 48], dt.float32)
        wrow = ppool.tile([128, NB, 128], dt.float32)
        agg = ppool.tile([128, NB, MSG], dt.float32)

        for rep in range(repeat):
            nc.sync.dma_start(
                s_nm[:], own0[:, 0:64].rearrange("(b p) j -> p b j", p=128))
            nc.vector.memset(v_nm[:], 0.0)
            nc.vector.memset(wrow[:], 0.0)
            # initial allgather: build tab0 from own0 slices
            wb0 = nc.sync.dma_start(cc_in[:], own0[:])
            cc0 = nc.gpsimd.collective_compute(
                "AllGather", mybir.AluOpType.bypass,
                replica_groups=[list(range(N_CORES))],
                ins=[cc_in[:]], outs=[tabs[0][:]])
            add_dep_helper(cc0.ins, wb0.ins, reason="ag0 waits own0 copy")
            prev_cc = cc0

            for l in range(LYR):
                tab = tabs[l]
                cur_group = [None]
                for c_i in range(cfg.n_chunks):
                    t0c = c_i * CH
                    gath = gpool.tile([128, CH, ROW], dt.float32)
                    gi = nc.gpsimd.dma_gather(
                        gath[:], tab[:], srcidx[:, t0c * 8:(t0c + CH) * 8],
                        CH * 128, CH * 128, ROW, single_packet=False)
                    if prev_cc is not None:
                        add_dep_helper(gi.ins, prev_cc.ins,
                                       reason="gather waits allgather")
                    tb = tpool.tile([128, CH, 128], dt.bfloat16)
                    nc.gpsimd.dma_start(tb[:], T_d[:, t0c:t0c + CH, :])
                    fb = fpool.tile([24, CH * 128], dt.bfloat16)
                    nc.sync.dma_start(fb[:], featT_d[:, t0c * 128:(t0c + CH) * 128])

                    msgt = mpool.tile([128, CH, MSG], dt.bfloat16)
                    for sc in range(n_sub):
                        pA = pgA.tile([128, SC, 128], dt.float32)
                        pB = pgB.tile([128, SC, 48], dt.float32)
                        for k in range(SC):
                            tg = t0c + sc * SC + k
                            si = cfg.set_of_tile(tg)
                            lhs = fb[:, (sc * SC + k) * 128:(sc * SC + k + 1) * 128]
                            nc.tensor.matmul(pA[:, k, 0:112], lhs,
                                             wfold[:, si * LYR + l, 0:112],
                                             start=True, stop=True)
                            nc.tensor.matmul(pB[:, k, :], lhs,
                                             wfold[:, si * LYR + l, 112:160],
                                             start=True, stop=True)
                        s8 = slice(sc * SC, (sc + 1) * SC)
                        rnbt = rpool.tile([128, SC, 3, 16], dt.bfloat16)
                        nc.scalar.activation(
                            rnbt[:],
                            rn_sb[:, t0c + sc * SC:t0c + (sc + 1) * SC, :]
                            .unsqueeze(3).broadcast_to([128, SC, 3, 16]),
                            AT.Copy)
                        nc.vector.tensor_tensor(
                            msgt[:, s8, 0:64], pA[:, :, 0:64],
                            gath[:, s8, 0:64], OP.mult)
                        nc.vector.tensor_tensor(
                            msgt[:, s8, 64:112], pA[:, :, 64:112],
                            gath[:, s8, 64:112], OP.mult)
                        tmp2 = rpool.tile([128, SC, 48], dt.bfloat16)
                        nc.vector.tensor_tensor(
                            tmp2[:], pB[:, :, :],
                            rnbt[:].rearrange("p a b c -> p a (b c)"), OP.mult)
                        nc.vector.tensor_tensor(
                            msgt[:, s8, 64:112], msgt[:, s8, 64:112],
                            tmp2[:], OP.add)

                    for k in range(CH):
                        tg = t0c + k
                        b = cfg.block_of_tile(tg)
                        si = cfg.set_of_tile(tg)
                        within = tg % cfg.tpb - (0 if si == 0 else TL)
                        TS = TL if si == 0 else TG
                        first, last = within == 0, within == TS - 1
                        if first:
                            cur_group[0] = pagg.tile([128, MSG], dt.float32,
                                                     name="paggt", tag="paggt")
                        nc.tensor.matmul(cur_group[0][:], tb[:, k, :],
                                         msgt[:, k, :], start=first, stop=last)
                        if last:
                            if si == 0:
                                nc.scalar.activation(
                                    agg[:, b, :], cur_group[0][:], AT.Copy,
                                    scale=invcnt[:, b, 0:1])
                            else:
                                nc.vector.scalar_tensor_tensor(
                                    agg[:, b, :], cur_group[0][:],
                                    invcnt[:, b, 1:2], agg[:, b, :],
                                    OP.mult, OP.add)

                # ---- node phase ----
                sT = npool.tile([64, NB, 128], dt.bfloat16, name="sT", tag="sT")
                aT = npool.tile([64, NB, 128], dt.bfloat16, name="aT", tag="aT")
                for b in range(NB):
                    ps1 = pnode.tile([64, 128], dt.float32, name="ps1", tag="pn")
                    nc.tensor.transpose(ps1[:], s_nm[:, b, :], idf[:])
                    nc.scalar.activation(sT[:, b, :], ps1[:], AT.Copy)
                    ps2 = pnode.tile([64, 128], dt.float32, name="ps2", tag="pn")
                    nc.tensor.transpose(ps2[:], agg[:, b, 0:64], idf[:])
                    nc.scalar.activation(aT[:, b, :], ps2[:], AT.Copy)
                h_sb = npool.tile([64, NPC], dt.bfloat16, name="h_sb", tag="h_sb")
                for chn in range(NPC // NCH):
                    ph = pnode.tile([64, NCH], dt.float32, name="ph", tag="pn")
                    sl = slice(chn * NCH, (chn + 1) * NCH)
                    nc.tensor.matmul(ph[:], wu1a[:, l, :],
                                     sT[:].rearrange("p a b -> p (a b)")[:, sl],
                                     start=True, stop=False)
                    nc.tensor.matmul(ph[:], wu1b[:, l, :],
                                     aT[:].rearrange("p a b -> p (a b)")[:, sl],
                                     start=False, stop=True)
                    nc.scalar.activation(h_sb[:, sl], ph[:], AT.Silu,
                                         bias=bu1[:, l, :])
                updT = npool.tile([64, NPC], dt.bfloat16, name="updT", tag="updT")
                for chn in range(NPC // NCH):
                    pu = pnode.tile([64, NCH], dt.float32, name="pu", tag="pn")
                    sl = slice(chn * NCH, (chn + 1) * NCH)
                    nc.tensor.matmul(pu[:], wu2[:, l, :], h_sb[:, sl],
                                     start=True, stop=True)
                    nc.vector.tensor_scalar_add(updT[:, sl], pu[:], bu2[:, l, :])
                hbuf = npool.tile([128, NB, 64], dt.float32, name="hbuf", tag="hbuf")
                for b in range(NB):
                    pt2 = pnode.tile([128, 64], dt.bfloat16, name="pt2", tag="pn")
                    nc.tensor.transpose(pt2[:], updT[:, b * 128:(b + 1) * 128],
                                        idb[0:64, 0:64])
                    nc.vector.tensor_tensor(hbuf[:, b, :], pt2[:],
                                            s_nm[:, b, :], OP.add)
                mu = npool.tile([128, NB, 1], dt.float32, name="mu", tag="mu")
                nc.vector.tensor_reduce(mu[:], hbuf[:], axis=mybir.AxisListType.X,
                                        op=OP.add)
                nc.vector.tensor_scalar_mul(mu[:], mu[:], 1.0 / 64.0)
                nc.vector.tensor_tensor(hbuf[:], hbuf[:],
                                        mu[:].broadcast_to([128, NB, 64]),
                                        OP.subtract)
                sq = npool.tile([128, NB, 64], dt.float32, name="sq", tag="sq")
                nc.vector.tensor_tensor(sq[:], hbuf[:], hbuf[:], OP.mult)
                var = npool.tile([128, NB, 1], dt.float32, name="var", tag="var")
                nc.vector.tensor_reduce(var[:], sq[:], axis=mybir.AxisListType.X,
                                        op=OP.add)
                std = npool.tile([128, NB, 1], dt.float32, name="std", tag="std")
                nc.scalar.activation(std[:], var[:], AT.Sqrt,
                                     scale=c_inv64[:], bias=c_eps5[:])
                rstd = npool.tile([128, NB, 1], dt.float32, name="rstd", tag="rstd")
                nc.vector.reciprocal(rstd[:], std[:])
                nc.vector.tensor_tensor(hbuf[:], hbuf[:],
                                        rstd[:].broadcast_to([128, NB, 64]),
                                        OP.mult)
                nc.vector.tensor_tensor(hbuf[:], hbuf[:],
                                        lng[:, l:l + 1, :].broadcast_to([128, NB, 64]),
                                        OP.mult)
                nc.vector.tensor_tensor(s_nm[:], hbuf[:],
                                        lnb[:, l:l + 1, :].broadcast_to([128, NB, 64]),
                                        OP.add)
                nc.vector.tensor_tensor(v_nm[:], v_nm[:], agg[:, :, 64:112],
                                        OP.add)

                if l < LYR - 1:
                    nc.vector.tensor_copy(wrow[:, :, 0:64], s_nm[:])
                    nc.vector.tensor_copy(wrow[:, :, 64:112], v_nm[:])
                    wb = nc.sync.dma_start(
                        cc_in[:].rearrange("(b p) j -> p b j", p=128), wrow[:])
                    cc = nc.gpsimd.collective_compute(
                        "AllGather", mybir.AluOpType.bypass,
                        replica_groups=[list(range(N_CORES))],
                        ins=[cc_in[:]], outs=[tabs[l + 1][:]])
                    add_dep_helper(cc.ins, wb.ins, reason="ag waits writeback")
                    prev_cc = cc

            # ---- readout ----
            sTr = npool.tile([64, NB, 128], dt.bfloat16, name="sTr", tag="sT")
            v3T = npool.tile([16, 3, NB, 128], dt.bfloat16, name="v3T", tag="v3T")
            for b in range(NB):
                ps1 = pnode.tile([64, 128], dt.float32, name="ps1", tag="pn")
                nc.tensor.transpose(ps1[:], s_nm[:, b, :], idf[:])
                nc.scalar.activation(sTr[:, b, :], ps1[:], AT.Copy)
                for i in range(3):
                    ps2 = pnode.tile([16, 128], dt.float32, name="ps2v", tag="pn")
                    nc.tensor.transpose(ps2[:], v_nm[:, b, i * 16:(i + 1) * 16],
                                        idf[:])
                    nc.scalar.activation(v3T[:, i, b, :], ps2[:], AT.Copy)
            vn2 = npool.tile([16, NPC], dt.float32, name="vn2", tag="vn2")
            sq_tmp = npool.tile([16, NPC], dt.float32, name="sq_tmp", tag="sq_tmp")
            v3f = v3T[:].rearrange("p a b c -> p (a b c)")
            for i in range(3):
                for chn in range(NPC // NCH):
                    pv = pnode.tile([16, NCH], dt.float32, name="pv", tag="pn")
                    sl = slice(chn * NCH, (chn + 1) * NCH)
                    nc.tensor.matmul(pv[:], wv[:],
                                     v3f[:, i * NPC + chn * NCH:i * NPC + (chn + 1) * NCH],
                                     start=True, stop=True)
                    if i == 0:
                        nc.scalar.activation(vn2[:, sl], pv[:], AT.Square)
                    else:
                        nc.scalar.activation(sq_tmp[:, sl], pv[:], AT.Square)
                if i > 0:
                    nc.vector.tensor_tensor(vn2[:], vn2[:], sq_tmp[:], OP.add)
            vn_sb = npool.tile([16, NPC], dt.bfloat16, name="vn_sb", tag="updT")
            nc.scalar.activation(vn_sb[:], vn2[:], AT.Sqrt, bias=c_eps8[0:16, :])
            hr = npool.tile([64, NPC], dt.bfloat16, name="hr", tag="h_sb")
            for chn in range(NPC // NCH):
                phr = pnode.tile([64, NCH], dt.float32, name="phr", tag="pn")
                sl = slice(chn * NCH, (chn + 1) * NCH)
                nc.tensor.matmul(phr[:], wd1a[:],
                                 sTr[:].rearrange("p a b -> p (a b)")[:, sl],
                                 start=True, stop=False)
                nc.tensor.matmul(phr[:], wd1b[:], vn_sb[:, sl],
                                 start=False, stop=True)
                nc.scalar.activation(hr[:, sl], phr[:], AT.Silu, bias=bd1[:])
            e_sb = npool.tile([1, NPC], dt.float32, name="e_sb", tag="hbuf")
            for chn in range(NPC // NCH):
                pe = pnode.tile([1, NCH], dt.float32, name="pe", tag="pn")
                sl = slice(chn * NCH, (chn + 1) * NCH)
                nc.tensor.matmul(pe[:], wd2[:], hr[:, sl], start=True, stop=True)
                nc.vector.tensor_scalar_add(e_sb[:, sl], pe[:], bd2[:])
            nc.sync.dma_start(e_out[:], e_sb[:])

    nc.compile()
    return nc




# ---------------------------------------------------------------------------
# PJRT runner (caches jit + device inputs)
# ---------------------------------------------------------------------------

class GnnRunner:
    def __init__(self, nc):
        import jax
        import numpy as _np
        from jax.sharding import Mesh, PartitionSpec
        from jax.experimental.shard_map import shard_map
        from concourse import mybir
        from concourse.bass2jax import (
            _bass_exec_p, install_neuronx_cc_hook, partition_id_tensor)
        self.jax = jax
        self.P = PartitionSpec
        install_neuronx_cc_hook()
        self.nc = nc
        pname = nc.partition_id_tensor.name if nc.partition_id_tensor else None
        in_names, out_names, out_avals, zero_outs = [], [], [], []
        for alloc in nc.m.functions[0].allocations:
            if not isinstance(alloc, mybir.MemoryLocationSet):
                continue
            name = alloc.memorylocations[0].name
            if alloc.kind == "ExternalInput":
                if name != pname:
                    in_names.append(name)
            elif alloc.kind == "ExternalOutput":
                out_names.append(name)
                shape = tuple(alloc.tensor_shape)
                dtype = mybir.dt.np(alloc.dtype)
                out_avals.append(jax.core.ShapedArray(shape, dtype))
                zero_outs.append(_np.zeros(shape, dtype))
        self.n_params = len(in_names)
        n_outs = len(out_avals)
        self.param_names = list(in_names)
        in_names = in_names + out_names
        if pname is not None:
            in_names.append(pname)
        self.out_names = out_names
        self.zero_outs = zero_outs
        donate = tuple(range(self.n_params, self.n_params + n_outs))

        def _body(*args):
            operands = list(args)
            if pname is not None:
                operands.append(partition_id_tensor())
            return tuple(_bass_exec_p.bind(
                *operands, out_avals=tuple(out_avals),
                in_names=tuple(in_names), out_names=tuple(out_names),
                lowering_input_output_aliases=(),
                sim_require_finite=True, sim_require_nnan=True, nc=nc))

        devices = jax.devices()[:N_CORES]
        self.mesh = Mesh(np.asarray(devices), ("core",))
        in_specs = (PartitionSpec("core"),) * (self.n_params + n_outs)
        out_specs = (PartitionSpec("core"),) * n_outs
        self.sharded = jax.jit(
            shard_map(_body, mesh=self.mesh, in_specs=in_specs,
                      out_specs=out_specs, check_rep=False),
            donate_argnums=donate, keep_unused=True)
        self.out_avals = out_avals
        self.dev_in = None

    def stage_inputs(self, in_maps):
        import jax
        from jax.sharding import NamedSharding
        concat = [np.concatenate([np.asarray(in_maps[c][n])
                                  for c in range(N_CORES)], axis=0)
                  for n in self.param_names]
        sh = NamedSharding(self.mesh, self.P("core"))
        self.dev_in = jax.device_put(concat, [sh] * len(concat))
        jax.block_until_ready(self.dev_in)

    def execute(self):
        import time
        import jax
        from jax.sharding import NamedSharding
        sh = NamedSharding(self.mesh, self.P("core"))
        zeros = [jax.device_put(
            np.zeros((N_CORES * z.shape[0], *z.shape[1:]), z.dtype), sh)
            for z in self.zero_outs]
        jax.block_until_ready(zeros)
        t0 = time.time()
        outs = self.sharded(*self.dev_in, *zeros)
        jax.block_until_ready(outs)
        dt = time.time() - t0
        res = [
            {n: np.asarray(outs[i]).reshape(N_CORES, *self.out_avals[i].shape)[c]
             for i, n in enumerate(self.out_names)}
            for c in range(N_CORES)
        ]
        return res, dt


# ---------------------------------------------------------------------------
_CACHE = {}


def kernel(**inputs):
    global LAST_HW_EXEC_NS
    import time as _time
    cfg = pick_cfg(inputs)
    G = np.asarray(inputs["t"]).shape[0]
    core_inputs, meta = prep(inputs, cfg)
    key = (cfg.NB, cfg.TL, cfg.TG, cfg.CH)
    if key not in _CACHE:
        nc = build(cfg)
        _CACHE[key] = GnnRunner(nc)
    r = _CACHE[key]
    in_maps = [{k: np.asarray(ci[k]) for k in INPUT_KEYS} for ci in core_inputs]
    r.stage_inputs(in_maps)
    res, dt1 = r.execute()       # compile+load on first call
    res, dt2 = r.execute()       # warm measurement
    LAST_HW_EXEC_NS = int(dt2 * 1e9)
    e_cores = [res[c]["e"][0] for c in range(N_CORES)]
    return epilogue(e_cores, meta, G).astype(np.float32)
